# revision 32
# baseline (speedup 1.0000x reference)
"""Trainium2 Bass kernel for nn_CountMeanOfFeatureInCluster.

Computation (one training-mode step of a VQ-codebook "count mean" module):
    assign[b] = argmin_c || x[b] - (m[c] - eps) ||_2        (B=8192, C=7, F=2048)
    counts[c], wsums[c]  = segment counts / sums of per-sample feature-sums
    scalar_mean[c] = wsums[c] / max(counts[c]*F, 1)
    out = where(counts > 32, 0.1*scalar_mean + 0.9*m, m)    # [7, 2048]

Distance argmin via  argmax_c ( <x_b, m'_c> - ||m'_c||^2/2 ),  m' = m - eps.

Data-parallel over 8 NeuronCores (1024 samples each, codebook replicated).
Per core, everything is fp8e4m3 (scores only pick an argmax; measured effect
on the final output is ~7e-5 relative — the output is 0.9*running_mean +
0.1*(sums/(counts*2048)), so per-sample score noise is crushed):

  SWDGE cast-DMA f32->fp8 (4 DMAs of 2 sample-tiles each)
  -> PE-transpose as BF16: each [128 samples, 128 bf16] block is a pair of
     fp8 features per element, so one 53ns transpose moves 256 features;
     a DMA pair (2 tiles, 16 transposes) fills one 2-bank PSUM tile.
     x-data fp8 bytes stay below 0x5E in magnitude, so a pair can never
     alias bf16 Inf/NaN; transpose mode streams an exact-1.0 identity.
  -> one uint32-bitcast PSUM->SBUF copy per pair (1/4 the element count =
     4x cheaper), split in half across DVE and ACT
  -> flipped matmul: the transposed tile holds feature-PAIR rows, so each
     128-feature chunk is covered by two parity matmuls whose stationary is
     a stride-2 fp8 view of the SBUF tile and whose streamed operand is the
     host-de-interleaved 8-column codebook (7 clusters + a ones column that
     yields per-sample feature sums), accumulating [128 samples, 8] raw f32
     scores in PSUM over 8 chunks x 2 parities
  -> per 4-tile group: DVE-copy the raw scores PSUM->SBUF and DMA them out.
     The HOST adds the -||m'_c||^2/2 bias, does the argmax, and reduces
     counts / weighted sums — removing the on-device argmax chain from the
     critical path entirely.

A run of tiny dummy transposes bridges the DMA-wait window so the PE
p-state ramp (mid->full clock after 3us of continuous busy) completes
before the real transposes start.
"""

import numpy as np

import concourse.bacc as bacc
import concourse.bass as bass
import concourse.mybir as mybir
import concourse.tile as tile
from concourse.alu_op_type import AluOpType
from concourse.bass_utils import run_bass_kernel_spmd

EPS = 1e-6
MOMENTUM = 0.1
C = 7
COUNT_THRESH = 32
B, F = 8192, 2048
NCORES = 8
BC = B // NCORES      # samples per core
NT = BC // 128        # 128-sample tiles per core (8)
NP = NT // 2          # tile pairs / cast DMAs (4)
GT = 4                # tiles per output group
NG = NT // GT         # groups (2)
FPC = F // 256        # bf16 feature-pair chunks per tile (8)
N_WARM = 30           # dummy transposes bridging the first-DMA wait
F32 = mybir.dt.float32
FP8 = mybir.dt.float8e4
BF16 = mybir.dt.bfloat16
U32 = mybir.dt.uint32

_cache: dict = {}


def _build_nc():
    nc = bacc.Bacc("TRN2", target_bir_lowering=False, debug=False)
    xs_ap = nc.dram_tensor("xs", [BC, F], F32, kind="ExternalInput").ap()
    # mtx[p, cc, par, n]: de-interleaved transposed codebook (fp8):
    # mtx[p, cc, par, n] = m'_aug[cc*256 + 2p + par, n], cols 0-6 = m'
    # = m - eps, col 7 = 1.0 (streams per-sample feature sums for free)
    mtx_ap = nc.dram_tensor(
        "mtx", [128, FPC * 2 * 8], FP8, kind="ExternalInput"
    ).ap()
    identb_ap = nc.dram_tensor("identb", [128, 128], BF16, kind="ExternalInput").ap()
    # raw scores per tile pair: [128 samples, 2 tiles, 7 ips + feature-sum]
    out_ap = nc.dram_tensor(
        "scores", [NT // 2, 128, 2 * 8], F32, kind="ExternalOutput"
    ).ap()

    # cast-DMA batches: three 2-tile DMAs then two 1-tile DMAs, so only a
    # single tile (not a pair) lands last on the serialized DMA track
    BATCH = (2, 2, 2, 1, 1)
    xs_pt = xs_ap.rearrange("(t p) f -> p t f", p=128)

    with tile.TileContext(nc) as tc:
        with (
            tc.tile_pool(name="const", bufs=1) as const_pool,
            tc.tile_pool(name="x", bufs=3) as x_pool,
            tc.tile_pool(name="xt", bufs=4) as xt_pool,
            tc.tile_pool(name="sb", bufs=2) as sb_pool,
            tc.tile_pool(name="ps_t", bufs=5, space="PSUM") as ps_t,
            tc.tile_pool(name="ps_v", bufs=1, space="PSUM") as ps_v,
            tc.tile_pool(name="ps_w", bufs=1, space="PSUM") as ps_w,
        ):
            # all 8 tiles' raw scores live in one PSUM bank: [128, NT, 8]
            vall = ps_v.tile([128, NT, 8], F32)

            # --- PE warmup: dummy transposes keep the PE continuously busy
            # through the first-DMA wait so the p-state ramp finishes before
            # real work arrives
            warm = const_pool.tile([128, 128], FP8)
            nc.vector.memset(warm[:], 0.0)
            wps = ps_w.tile([128, 128, 2], FP8)
            for _ in range(N_WARM):
                nc.tensor.transpose(wps[:, :, 0], warm[:], warm[:])
            # dummy ACT op: absorb the one-time activation-table load (1.3us)
            # before the first real PSUM->SBUF copy needs the engine
            wsb = const_pool.tile([1, 1], F32)
            nc.scalar.copy(wsb[:], warm[0:1, 0:4].bitcast(F32))

            # --- constants
            mtx_t = const_pool.tile([128, FPC, 2, 8], FP8)
            nc.sync.dma_start(
                mtx_t[:].rearrange("p a b c -> p (a b c)"), mtx_ap[:]
            )
            identb_t = const_pool.tile([128, 128], BF16)
            nc.sync.dma_start(identb_t[:], identb_ap[:])

            # --- prefetch all cast-DMAs (SWDGE: f32 DRAM -> fp8 SBUF)
            xtiles = []       # per-tile [128, F] fp8 views
            t0 = 0
            for nb in BATCH:
                xd = x_pool.tile([128, nb, F], FP8, tag=f"x{nb}")
                nc.gpsimd.dma_start(xd[:], xs_pt[:, t0:t0 + nb, :])
                for q in range(nb):
                    xtiles.append(xd[:, q, :])
                t0 += nb

            def emit_transposes(t):
                tp = ps_t.tile([128, FPC, 128], BF16, tag="tp")
                xb = xtiles[t].bitcast(BF16)
                for cc in range(FPC):
                    nc.tensor.transpose(
                        tp[:, cc, :], xb[:, cc * 128:(cc + 1) * 128], identb_t[:]
                    )
                return tp

            def emit_copy(t, tp, split=False):
                xt = xt_pool.tile([128, FPC, 128], BF16, tag="xt")
                src = tp[:].rearrange("p a b -> p (a b)").bitcast(U32)
                dst = xt[:].rearrange("p a b -> p (a b)").bitcast(U32)
                h = FPC * 32
                if split:
                    # last tile: halve the exposed latency
                    nc.vector.tensor_copy(dst[:, 0:h], src[:, 0:h])
                    nc.scalar.copy(dst[:, h:], src[:, h:])
                elif t % 2 == 0:
                    nc.vector.tensor_copy(dst[:], src[:])
                else:
                    nc.scalar.copy(dst[:], src[:])
                return xt

            def emit_matmuls(t, xt):
                # partition p of chunk cc holds features (cc*256+2p, +1);
                # two parity matmuls per chunk with the de-interleaved mtx
                v = xt[:].rearrange("p a b -> p (a b)").bitcast(FP8).rearrange(
                    "p (a b c) -> p a b c", a=FPC, b=128, c=2
                )
                for cc in range(FPC):
                    for par in range(2):
                        nc.tensor.matmul(
                            vall[:, t, :],
                            lhsT=v[:, cc, :, par],
                            rhs=mtx_t[:, cc, par, :],
                            start=(cc == 0 and par == 0),
                            stop=(cc == FPC - 1 and par == 1),
                        )

            def emit_pair_out(j):
                # DMA raw scores of tiles 2j, 2j+1 as soon as they stop
                sv = sb_pool.tile([128, 2, 8], F32, tag="sv")
                nc.vector.tensor_copy(sv[:], vall[:, 2 * j:2 * j + 2, :])
                nc.sync.dma_start(
                    out_ap[j], sv[:].rearrange("p q n -> p (q n)")
                )

            # software pipeline, depth 2: matmuls(t) are emitted after
            # transposes(t+2) so the PE never waits on tile t's copy
            tps, xts = {}, {}
            for t in range(NT):
                tps[t] = emit_transposes(t)
                xts[t] = emit_copy(t, tps[t], split=(t == NT - 1))
                if t >= 2:
                    emit_matmuls(t - 2, xts[t - 2])
                    if (t - 2) % 2 == 1:
                        emit_pair_out((t - 2) // 2)
            emit_matmuls(NT - 2, xts[NT - 2])
            emit_matmuls(NT - 1, xts[NT - 1])
            emit_pair_out(NT // 2 - 2)
            emit_pair_out(NT // 2 - 1)

    nc.compile()
    return nc


def _get_nc():
    if "nc" not in _cache:
        _cache["nc"] = _build_nc()
    return _cache["nc"]


def _fp8_np():
    import ml_dtypes

    return np.dtype(ml_dtypes.float8_e4m3)


def _host_inputs(running_mean: np.ndarray):
    import ml_dtypes

    mp = running_mean.astype(np.float64) - EPS           # [C, F]
    mt_aug = np.zeros((F, 8), dtype=np.float64)
    mt_aug[:, :C] = mp.T
    mt_aug[:, C] = 1.0
    mt_q = mt_aug.astype(_fp8_np())
    # de-interleaved chunk layout: mtx[p, cc, par, n] = mt_q[cc*256+2p+par, n]
    mtx = np.ascontiguousarray(
        mt_q.reshape(FPC, 128, 2, 8).transpose(1, 0, 2, 3).reshape(128, FPC * 2 * 8)
    )
    # bias matches what the PE actually multiplies: the fp8-rounded m'
    mpq = mt_q[:, :C].astype(np.float64)
    hb = (-0.5 * (mpq * mpq).sum(axis=0)).astype(np.float32)       # [C]
    identb = np.eye(128).astype(np.dtype(ml_dtypes.bfloat16))
    return mtx, hb, identb


def kernel(x: np.ndarray, running_mean: np.ndarray) -> np.ndarray:
    x = np.asarray(x, dtype=np.float32)
    running_mean = np.asarray(running_mean, dtype=np.float32)
    nc = _get_nc()
    mtx, hb, identb = _host_inputs(running_mean)
    in_maps = [
        {
            "xs": np.ascontiguousarray(x[i * BC:(i + 1) * BC]),
            "mtx": mtx,
            "identb": identb,
        }
        for i in range(NCORES)
    ]
    res = run_bass_kernel_spmd(nc, in_maps, core_ids=list(range(NCORES)))
    counts = np.zeros(C, dtype=np.int64)
    wsums = np.zeros(C, dtype=np.float64)
    for r in res.results:
        # scores[j, p, q, n] -> sample (2j+q)*128 + p, raw ip / feature sum
        s = r["scores"].reshape(NT // 2, 128, 2, 8)
        s = s.transpose(0, 2, 1, 3).reshape(BC, 8)
        assign = np.argmax(s[:, :C] + hb[None, :], axis=1)
        counts += np.bincount(assign, minlength=C)
        wsums += np.bincount(assign, weights=s[:, C].astype(np.float64),
                             minlength=C)
    counts_f = counts.astype(np.float32)
    scalar_mean = (wsums.astype(np.float32)
                   / np.maximum(counts_f * np.float32(F), np.float32(1.0)))
    update = (np.float32(MOMENTUM) * scalar_mean)[:, None] + np.float32(
        1.0 - MOMENTUM
    ) * running_mean
    out = np.where((counts_f > COUNT_THRESH)[:, None], update, running_mean)
    return out.astype(np.float32)
